# revision 2
# baseline (speedup 1.0000x reference)
"""DiceLoss kernel for Trainium2 (raw Bass, no Tile), 8-core data parallel.

Problem: predict/target [2, 4, 64, 256, 256] f32.
  p = sigmoid(predict); per (b, o, d) slice of 65536 elements:
    num = sum(p*t), den = sum(p) + sum(t) + 1
    dice = 1 - 2*num/den
  per-(b,o) mean over valid d slices, then mean over the 8 (b,o) pairs.

The baseline (f32 in HBM, per-slice [128, 512] ops) sat exactly on the
f32 DMA roofline (32 MiB/core at ~360 GB/s ~ 92 us). This version cuts
HBM bytes and instruction overheads:

* dtypes: predict is cast host-side to fp8 e4m3 (TRN FP8_EXP4; exact
  encoding match to ml_dtypes.float8_e4m3 for |x|<=240), target to fp16.
  12 MiB/core -> ~35 us DMA floor. Accuracy: sigmoid is 0.25-Lipschitz,
  fp8 rounding of N(0,1) inputs is zero-mean to first order, and each
  dice term averages 65536 elements, so the scalar output error lands
  ~1e-3 (tolerance 2e-2). All sums accumulate in f32 on device.

* slice-stacked layout: one chunk = [128, 4096] holding 8 slices, slice
  j on partitions [16j, 16j+16), 4096 of its elements per partition.
  accum_out [128, 1] per op still yields per-slice partials (host sums
  each 16-partition group), but every engine op covers 8 slices, so the
  per-instruction overheads (ACT 224 cyc, DVE 58 cyc) amortize 8x.

* product in two DVE passes instead of one fused scalar_tensor_tensor:
  stt supports NO DVE perf modes (1x always), while tensor_tensor runs
  2x_1p on fp16 and tensor_scalar runs 4x_2p. tt(mult) -> prod, then
  ts(*1.0, accum_out) over prod. sum(t) is a 4x ts over the target
  chunk; the last 2 chunks' sum(t) run on ACT (Copy+accum after the
  sigmoids) to balance engine time (DVE ~33 us, ACT ~36 us vs the
  ~35 us DMA floor).

Engine budget per core (8 chunks): ACT sigmoid+accum (224+4096)/1.2GHz
= 3.6 us * 8 + 2 Copy-accum = 36 us; DVE tt (58+2048)/0.96 = 2.2 us +
ts-prod (58+1024)/0.96 = 1.13 + ts-tgt 1.13 -> ~33 us.

The dummy `out` of the ts accumulation passes is written over the sig
buffer slot the chunk just consumed (same-engine ordering; the next ACT
writer already waits on dve_sem), so no extra scratch is needed.

The [128, 3, 8] accumulator tile is DMA'd out once; host does the
16-partition-group sums and the tiny dice math over 512 slices.
"""

from contextlib import ExitStack

import numpy as np

import concourse.bass as bass
from concourse import mybir
from concourse.bass_utils import run_bass_kernel_spmd

N_CORES = 8
B, O, D = 2, 4, 64
HW = 256 * 256              # elements per slice
P = 128                     # SBUF partitions
S = (B * O * D) // N_CORES  # 64 slices per core (= one (b,o) pair)
R = 8                       # slices stacked per chunk
PPS = P // R                # 16 partitions per slice
NCHUNK = S // R             # 8 chunks per core
FD = HW // PPS              # 4096 free elems per partition per chunk
NSLOT = 3                   # DMA buffer slots per stream
SIG_BUFS = 2                # sigmoid output slots
PROD_BUFS = 2               # product scratch slots
ACT_T = (6, 7)              # chunks whose sum(t) runs on ACT (after sigmoids)
SMOOTH = 1.0

PRED_DT = mybir.dt.float8e4     # fp8 e4m3 on the wire
TGT_DT = mybir.dt.float16
f32 = mybir.dt.float32
AF = mybir.ActivationFunctionType
ALU = mybir.AluOpType


def build_nc(repeats=1):
    """Build the per-core Bass program (same program on all cores).

    repeats > 1 re-runs the whole body that many times (re-reading the
    same DRAM) — used only for slope-based wall-clock timing."""
    total = repeats * NCHUNK

    nc = bass.Bass("TRN2", debug=False, enable_asserts=False)

    pred = nc.dram_tensor("pred", [P, NCHUNK, FD], PRED_DT,
                          kind="ExternalInput").ap()
    tgt = nc.dram_tensor("tgt", [P, NCHUNK, FD], TGT_DT,
                         kind="ExternalInput").ap()
    # out_acc[:, 0, c] = sum(p), [:, 1, c] = sum(t), [:, 2, c] = sum(p*t)
    out_acc = nc.dram_tensor("out_acc", [P, 3, NCHUNK], f32,
                             kind="ExternalOutput").ap()

    # actt count completed once chunk g's ACT Copy-accum is done
    def actt_done(g):
        r, c = divmod(g, NCHUNK)
        assert c in ACT_T
        return r * len(ACT_T) + ACT_T.index(c) + 1

    with ExitStack() as ctx:
        pred_buf = ctx.enter_context(nc.sbuf_tensor([P, NSLOT, FD], PRED_DT))
        tgt_buf = ctx.enter_context(nc.sbuf_tensor([P, NSLOT, FD], TGT_DT))
        sig_buf = ctx.enter_context(nc.sbuf_tensor([P, SIG_BUFS, FD], TGT_DT))
        prod_buf = ctx.enter_context(nc.sbuf_tensor([P, PROD_BUFS, FD], TGT_DT))
        scr_a = ctx.enter_context(nc.sbuf_tensor([P, len(ACT_T), FD], TGT_DT))
        acc = ctx.enter_context(nc.sbuf_tensor([P, 3, NCHUNK], f32))
        # One DMA sem per buffer slot: at most one load in flight per sem,
        # so "sem >= 16*uses" proves that load is complete.
        dma_p = [ctx.enter_context(nc.semaphore(f"dma_p{i}"))
                 for i in range(NSLOT)]
        dma_t = [ctx.enter_context(nc.semaphore(f"dma_t{i}"))
                 for i in range(NSLOT)]
        sig_sem = ctx.enter_context(nc.semaphore("sig_sem"))    # +1/sigmoid
        actt_sem = ctx.enter_context(nc.semaphore("actt_sem"))  # +1/ACT copy
        dve_sem = ctx.enter_context(nc.semaphore("dve_sem"))    # +1/chunk
        out_sem = ctx.enter_context(nc.semaphore("out_sem"))
        block = ctx.enter_context(nc.Block())

        sp_acc = acc[:, 0, :]
        st_acc = acc[:, 1, :]
        spt_acc = acc[:, 2, :]

        @block.sync
        def _(sync):
            for g in range(total):
                c = g % NCHUNK
                slot = g % NSLOT
                if g >= NSLOT:
                    pg = g - NSLOT  # previous user of this slot
                    # pred slot: ACT sigmoid of pg done
                    sync.wait_ge(sig_sem, pg + 1)
                    # tgt slot: DVE ops of pg done (+ ACT copy if assigned)
                    sync.wait_ge(dve_sem, pg + 1)
                    if pg % NCHUNK in ACT_T:
                        sync.wait_ge(actt_sem, actt_done(pg))
                sync.dma_start(pred_buf[:, slot, :], pred[:, c]
                               ).then_inc(dma_p[slot], 16)
                sync.dma_start(tgt_buf[:, slot, :], tgt[:, c]
                               ).then_inc(dma_t[slot], 16)
            sync.wait_ge(sig_sem, total)
            sync.wait_ge(dve_sem, total)
            sync.wait_ge(actt_sem, repeats * len(ACT_T))
            sync.dma_start(out_acc, acc[:]).then_inc(out_sem, 16)
            sync.wait_ge(out_sem, 16)

        @block.scalar
        def _(scalar):
            for r in range(repeats):
                for c in range(NCHUNK):
                    g = r * NCHUNK + c
                    slot = g % NSLOT
                    sslot = g % SIG_BUFS
                    scalar.wait_ge(dma_p[slot], 16 * (g // NSLOT + 1))
                    if g >= SIG_BUFS:
                        # sig slot free once DVE finished chunk g-2
                        scalar.wait_ge(dve_sem, g - 1)
                    nc.scalar.activation(
                        sig_buf[:, sslot, :], pred_buf[:, slot, :],
                        AF.Sigmoid, accum_out=sp_acc[:, c:c + 1],
                    ).then_inc(sig_sem, 1)
                # tail: sum(t) for the ACT-assigned chunks of this repeat
                for k, c in enumerate(ACT_T):
                    g = r * NCHUNK + c
                    slot = g % NSLOT
                    scalar.wait_ge(dma_t[slot], 16 * (g // NSLOT + 1))
                    if r >= 1:
                        # scr_a slot WAW vs previous repeat; same-engine
                        # order, wait is an already-passed proof
                        scalar.wait_ge(actt_sem, (r - 1) * len(ACT_T) + k + 1)
                    nc.scalar.activation(
                        scr_a[:, k, :], tgt_buf[:, slot, :], AF.Copy,
                        accum_out=st_acc[:, c:c + 1],
                    ).then_inc(actt_sem, 1)

        @block.vector
        def _(vector):
            for g in range(total):
                c = g % NCHUNK
                slot = g % NSLOT
                sslot = g % SIG_BUFS
                pslot = g % PROD_BUFS
                vector.wait_ge(sig_sem, g + 1)
                vector.wait_ge(dma_t[slot], 16 * (g // NSLOT + 1))
                if g >= 2:
                    # prod slot & sig-slot dummy-out WAW vs chunk g-2;
                    # already satisfied (same engine), race-proof only
                    vector.wait_ge(dve_sem, g - 1)
                nc.vector.tensor_tensor(
                    out=prod_buf[:, pslot, :],
                    in0=sig_buf[:, sslot, :],
                    in1=tgt_buf[:, slot, :],
                    op=ALU.mult,
                )
                ts_prod = nc.vector.tensor_scalar(
                    out=sig_buf[:, sslot, :], in0=prod_buf[:, pslot, :],
                    scalar1=1.0, scalar2=None,
                    op0=ALU.mult, op1=ALU.add,
                    accum_out=spt_acc[:, c:c + 1],
                )
                if c in ACT_T:
                    ts_prod.then_inc(dve_sem, 1)
                else:
                    nc.vector.tensor_scalar(
                        out=sig_buf[:, sslot, :], in0=tgt_buf[:, slot, :],
                        scalar1=1.0, scalar2=None,
                        op0=ALU.mult, op1=ALU.add,
                        accum_out=st_acc[:, c:c + 1],
                    ).then_inc(dve_sem, 1)

    return nc


_NC_CACHE = {}


def _get_nc():
    if "nc" not in _NC_CACHE:
        _NC_CACHE["nc"] = build_nc()
    return _NC_CACHE["nc"]


def _shard_one(x, np_dt):
    """[S, HW] f32 -> [128, NCHUNK, FD] in np_dt, slice-stacked layout:
    chunk c holds slices 8c..8c+7, slice j of a chunk on partitions
    [16j, 16j+16), 4096 consecutive elements per partition."""
    v = x.reshape(NCHUNK, R, PPS, FD)          # (c, j, q, f)
    v = v.transpose(1, 2, 0, 3)                # (j, q, c, f)
    return np.ascontiguousarray(v.reshape(P, NCHUNK, FD).astype(np_dt))


def shard_inputs(predict, target):
    pred_np = mybir.dt.np(PRED_DT)
    tgt_np = mybir.dt.np(TGT_DT)
    pred_sh = np.asarray(predict, dtype=np.float32).reshape(N_CORES, S, HW)
    tgt_sh = np.asarray(target, dtype=np.float32).reshape(N_CORES, S, HW)
    return [
        {"pred": _shard_one(pred_sh[i], pred_np),
         "tgt": _shard_one(tgt_sh[i], tgt_np)}
        for i in range(N_CORES)
    ]


def finish(results, target):
    """Host-side: 16-partition-group sums of the [128, 3, NCHUNK]
    partials + dice math over the 512 slices."""
    sp = np.empty((N_CORES, S), np.float64)
    st = np.empty((N_CORES, S), np.float64)
    spt = np.empty((N_CORES, S), np.float64)
    for i, res in enumerate(results):
        a = res["out_acc"].astype(np.float64)
        a = a.reshape(R, PPS, 3, NCHUNK).sum(axis=1)   # [j, 3, c]
        # slice s = 8c + j  ->  order (c, j)
        sp[i] = a[:, 0, :].T.reshape(S)
        st[i] = a[:, 1, :].T.reshape(S)
        spt[i] = a[:, 2, :].T.reshape(S)

    dice = 1.0 - 2.0 * spt / (sp + st + SMOOTH)          # [B*O, D]
    tfirst = np.asarray(target, dtype=np.float32).reshape(B * O, D, HW)[:, :, 0]
    valid = (tfirst != -1.0).astype(np.float64)
    per_pair = (dice * valid).sum(axis=-1) / valid.sum(axis=-1)  # [B*O]
    return np.array(per_pair.mean(), dtype=np.float32)


def kernel(predict: np.ndarray, target: np.ndarray) -> np.ndarray:
    predict = np.asarray(predict)
    target = np.asarray(target)
    assert predict.shape == (B, O, D, 256, 256)
    in_maps = shard_inputs(predict, target)
    nc = _get_nc()
    res = run_bass_kernel_spmd(nc, in_maps, list(range(N_CORES)))
    return finish(res.results, target)


# revision 5
# speedup vs baseline: 1.1596x; 1.1596x over previous
"""DiceLoss kernel for Trainium2 (raw Bass, no Tile), 8-core data parallel.

Problem: predict/target [2, 4, 64, 256, 256] f32.
  p = sigmoid(predict); per (b, o, d) slice of 65536 elements:
    num = sum(p*t), den = sum(p) + sum(t) + 1
    dice = 1 - 2*num/den
  per-(b,o) mean over valid d slices, then mean over the 8 (b,o) pairs.

The baseline (f32 in HBM, per-slice [128, 512] ops) sat exactly on the
f32 DMA roofline (32 MiB/core at ~360 GB/s ~ 92 us). This version cuts
HBM bytes and instruction overheads:

* dtypes: predict is cast host-side to fp8 e4m3 (TRN FP8_EXP4; exact
  encoding match to ml_dtypes.float8_e4m3 for |x|<=240), target to fp16.
  12 MiB/core -> ~35 us DMA floor. Accuracy: sigmoid is 0.25-Lipschitz,
  fp8 rounding of N(0,1) inputs is zero-mean to first order, and each
  dice term averages 65536 elements, so the scalar output error lands
  ~1e-3 (tolerance 2e-2). All sums accumulate in f32 on device.

* slice-stacked layout: one chunk = [128, 4096] holding 8 slices, slice
  j on partitions [16j, 16j+16), 4096 of its elements per partition.
  accum_out [128, 1] per op still yields per-slice partials (host sums
  each 16-partition group), but every engine op covers 8 slices, so the
  per-instruction overheads (ACT 224 cyc, DVE 58 cyc) amortize 8x.

* product in two DVE passes instead of one fused scalar_tensor_tensor:
  stt supports NO DVE perf modes (1x always), while tensor_tensor runs
  2x_1p on fp16 and tensor_scalar runs 4x_2p. tt(mult) -> prod, then
  ts(*1.0, accum_out) over prod. sum(t) is a 4x ts over the target
  chunk; the last 2 chunks' sum(t) run on ACT (Copy+accum after the
  sigmoids) to balance engine time (DVE ~33 us, ACT ~36 us vs the
  ~35 us DMA floor).

Engine budget per core (8 chunks): ACT sigmoid+accum (224+4096)/1.2GHz
= 3.6 us * 8 + 2 Copy-accum = 36 us; DVE tt (58+2048)/0.96 = 2.2 us +
ts-prod (58+1024)/0.96 = 1.13 + ts-tgt 1.13 -> ~33 us.

The dummy `out` of the ts accumulation passes is written over the sig
buffer slot the chunk just consumed (same-engine ordering; the next ACT
writer already waits on dve_sem), so no extra scratch is needed.

The [128, 3, 8] accumulator tile is DMA'd out once; host does the
16-partition-group sums and the tiny dice math over 512 slices.
"""

from contextlib import ExitStack

import numpy as np

import concourse.bass as bass
from concourse import mybir
from concourse.bass_utils import run_bass_kernel_spmd

N_CORES = 8
B, O, D = 2, 4, 64
HW = 256 * 256              # elements per slice
P = 128                     # SBUF partitions
S = (B * O * D) // N_CORES  # 64 slices per core (= one (b,o) pair)
R = 8                       # slices stacked per chunk
PPS = P // R                # 16 partitions per slice
NCHUNK = S // R             # 8 chunks per core
FD = HW // PPS              # 4096 free elems per partition per chunk
NSLOT = 3                   # DMA buffer slots per stream
SIG_BUFS = 2                # sigmoid output slots
PROD_BUFS = 2               # product scratch slots
SMOOTH = 1.0

PRED_DT = mybir.dt.float8e4     # fp8 e4m3 on the wire
TGT_DT = mybir.dt.float16
f32 = mybir.dt.float32
AF = mybir.ActivationFunctionType
ALU = mybir.AluOpType


def build_nc(repeats=1):
    """Build the per-core Bass program (same program on all cores).

    repeats > 1 re-runs the whole body that many times (re-reading the
    same DRAM) — used only for slope-based wall-clock timing."""
    total = repeats * NCHUNK

    nc = bass.Bass("TRN2", debug=False, enable_asserts=False)

    pred = nc.dram_tensor("pred", [P, NCHUNK, FD], PRED_DT,
                          kind="ExternalInput").ap()
    tgt = nc.dram_tensor("tgt", [P, NCHUNK, FD], TGT_DT,
                         kind="ExternalInput").ap()
    # out_acc[:, 0, c] = sum(p), [:, 1, c] = sum(t), [:, 2, c] = sum(p*t)
    out_acc = nc.dram_tensor("out_acc", [P, 3, NCHUNK], f32,
                             kind="ExternalOutput").ap()

    with ExitStack() as ctx:
        pred_buf = ctx.enter_context(nc.sbuf_tensor([P, NSLOT, FD], PRED_DT))
        tgt_buf = ctx.enter_context(nc.sbuf_tensor([P, NSLOT, FD], TGT_DT))
        sig_buf = ctx.enter_context(nc.sbuf_tensor([P, SIG_BUFS, FD], TGT_DT))
        prod_buf = ctx.enter_context(nc.sbuf_tensor([P, PROD_BUFS, FD], TGT_DT))
        scr_a = ctx.enter_context(nc.sbuf_tensor([P, 2, FD], TGT_DT))
        acc = ctx.enter_context(nc.sbuf_tensor([P, 3, NCHUNK], f32))
        # One DMA sem per buffer slot: at most one load in flight per sem,
        # so "sem >= 16*uses" proves that load is complete.
        dma_p = [ctx.enter_context(nc.semaphore(f"dma_p{i}"))
                 for i in range(NSLOT)]
        dma_t = [ctx.enter_context(nc.semaphore(f"dma_t{i}"))
                 for i in range(NSLOT)]
        sig_sem = ctx.enter_context(nc.semaphore("sig_sem"))    # +1/sigmoid
        actt_sem = ctx.enter_context(nc.semaphore("actt_sem"))  # +1/ACT copy
        dve_sem = ctx.enter_context(nc.semaphore("dve_sem"))    # +1/chunk
        out_sem = ctx.enter_context(nc.semaphore("out_sem"))
        block = ctx.enter_context(nc.Block())

        sp_acc = acc[:, 0, :]
        st_acc = acc[:, 1, :]
        spt_acc = acc[:, 2, :]

        @block.sync
        def _(sync):
            for g in range(total):
                c = g % NCHUNK
                slot = g % NSLOT
                if g >= NSLOT:
                    pg = g - NSLOT  # previous user of this slot
                    # pred slot: ACT sigmoid of pg done
                    sync.wait_ge(sig_sem, pg + 1)
                    # tgt slot: DVE tt of pg done (implied by its ts-prod
                    # inc) and ACT copy of pg done
                    sync.wait_ge(dve_sem, pg + 1)
                    sync.wait_ge(actt_sem, pg + 1)
                sync.dma_start(pred_buf[:, slot, :], pred[:, c]
                               ).then_inc(dma_p[slot], 16)
                sync.dma_start(tgt_buf[:, slot, :], tgt[:, c]
                               ).then_inc(dma_t[slot], 16)
            sync.wait_ge(sig_sem, total)
            sync.wait_ge(dve_sem, total)
            sync.wait_ge(actt_sem, total)
            sync.dma_start(out_acc, acc[:]).then_inc(out_sem, 16)
            sync.wait_ge(out_sem, 16)

        @block.scalar
        def _(scalar):
            for g in range(total):
                c = g % NCHUNK
                slot = g % NSLOT
                sslot = g % SIG_BUFS
                scalar.wait_ge(dma_p[slot], 16 * (g // NSLOT + 1))
                if g >= SIG_BUFS:
                    # sig slot free once DVE finished chunk g-2
                    scalar.wait_ge(dve_sem, g - 1)
                nc.scalar.activation(
                    sig_buf[:, sslot, :], pred_buf[:, slot, :],
                    AF.Sigmoid, accum_out=sp_acc[:, c:c + 1],
                ).then_inc(sig_sem, 1)
                # sum(t) for this chunk (Copy shares the sigmoid ACT table)
                scalar.wait_ge(dma_t[slot], 16 * (g // NSLOT + 1))
                if g >= 2:
                    # scr_a slot WAW vs chunk g-2; same-engine order,
                    # wait is an already-passed proof
                    scalar.wait_ge(actt_sem, g - 1)
                nc.scalar.activation(
                    scr_a[:, g % 2, :], tgt_buf[:, slot, :], AF.Copy,
                    accum_out=st_acc[:, c:c + 1],
                ).then_inc(actt_sem, 1)

        @block.vector
        def _(vector):
            for g in range(total):
                c = g % NCHUNK
                slot = g % NSLOT
                sslot = g % SIG_BUFS
                pslot = g % PROD_BUFS
                vector.wait_ge(sig_sem, g + 1)
                vector.wait_ge(dma_t[slot], 16 * (g // NSLOT + 1))
                if g >= 2:
                    # prod slot & sig-slot dummy-out WAW vs chunk g-2;
                    # already satisfied (same engine), race-proof only
                    vector.wait_ge(dve_sem, g - 1)
                nc.vector.tensor_tensor(
                    out=prod_buf[:, pslot, :],
                    in0=sig_buf[:, sslot, :],
                    in1=tgt_buf[:, slot, :],
                    op=ALU.mult,
                )
                nc.vector.tensor_scalar(
                    out=sig_buf[:, sslot, :], in0=prod_buf[:, pslot, :],
                    scalar1=1.0, scalar2=None,
                    op0=ALU.mult, op1=ALU.add,
                    accum_out=spt_acc[:, c:c + 1],
                ).then_inc(dve_sem, 1)

    return nc


_NC_CACHE = {}


def _get_nc():
    if "nc" not in _NC_CACHE:
        _NC_CACHE["nc"] = build_nc()
    return _NC_CACHE["nc"]


def _shard_one(x, np_dt):
    """[S, HW] f32 -> [128, NCHUNK, FD] in np_dt, slice-stacked layout:
    chunk c holds slices 8c..8c+7, slice j of a chunk on partitions
    [16j, 16j+16), 4096 consecutive elements per partition."""
    v = x.reshape(NCHUNK, R, PPS, FD)          # (c, j, q, f)
    v = v.transpose(1, 2, 0, 3)                # (j, q, c, f)
    return np.ascontiguousarray(v.reshape(P, NCHUNK, FD).astype(np_dt))


def shard_inputs(predict, target):
    pred_np = mybir.dt.np(PRED_DT)
    tgt_np = mybir.dt.np(TGT_DT)
    pred_sh = np.asarray(predict, dtype=np.float32).reshape(N_CORES, S, HW)
    tgt_sh = np.asarray(target, dtype=np.float32).reshape(N_CORES, S, HW)
    return [
        {"pred": _shard_one(pred_sh[i], pred_np),
         "tgt": _shard_one(tgt_sh[i], tgt_np)}
        for i in range(N_CORES)
    ]


def finish(results, target):
    """Host-side: 16-partition-group sums of the [128, 3, NCHUNK]
    partials + dice math over the 512 slices."""
    sp = np.empty((N_CORES, S), np.float64)
    st = np.empty((N_CORES, S), np.float64)
    spt = np.empty((N_CORES, S), np.float64)
    for i, res in enumerate(results):
        a = res["out_acc"].astype(np.float64)
        a = a.reshape(R, PPS, 3, NCHUNK).sum(axis=1)   # [j, 3, c]
        # slice s = 8c + j  ->  order (c, j)
        sp[i] = a[:, 0, :].T.reshape(S)
        st[i] = a[:, 1, :].T.reshape(S)
        spt[i] = a[:, 2, :].T.reshape(S)

    dice = 1.0 - 2.0 * spt / (sp + st + SMOOTH)          # [B*O, D]
    tfirst = np.asarray(target, dtype=np.float32).reshape(B * O, D, HW)[:, :, 0]
    valid = (tfirst != -1.0).astype(np.float64)
    per_pair = (dice * valid).sum(axis=-1) / valid.sum(axis=-1)  # [B*O]
    return np.array(per_pair.mean(), dtype=np.float32)


def kernel(predict: np.ndarray, target: np.ndarray) -> np.ndarray:
    predict = np.asarray(predict)
    target = np.asarray(target)
    assert predict.shape == (B, O, D, 256, 256)
    in_maps = shard_inputs(predict, target)
    nc = _get_nc()
    res = run_bass_kernel_spmd(nc, in_maps, list(range(N_CORES)))
    return finish(res.results, target)
